# revision 12
# baseline (speedup 1.0000x reference)
"""GatedGCN layer on 8 Trainium2 NeuronCores (Bass/Tile).

Sharding (per the problem's hint): nodes are partitioned into 8 contiguous
ranges; each core owns the edges whose dst is in its range, so the segment-sum
scatter is core-local. The host routes each edge to its dst core together with
its payload (e row, src id, and - in halo mode - the boundary src node's raw
h/norm features, i.e. the materialized "all-gather of boundary src features"
from the hint). Edges are dst-sorted and padded so each node occupies a fixed
W-slot group (W = degree class); segment sums then become fixed-window Vector
engine reductions in a feature-major layout [128 = 4 blocks x 32 features,
columns]. h_dst is free via stride-0 broadcast APs over a resident per-core
node table. All arithmetic of the reference (h*norm - including for the
halo-replicated src copies - gate, sigmoid, weighted mean, batch-norm, relu)
runs on device. BatchNorm statistics ride the Scalar engine's accumulator
(pad slots excluded exactly via a 0/1 mask), are folded by a tiny PE matmul,
AllReduce'd across the 8 cores, and applied as one Relu-affine activation per
tile. e_ij is staged to DRAM in f32 between the stats pass and the output
pass.

GATHER_MODE:
  "halo"   - src features are routed with the edges by the host (fast path).
  "device" - src features are gathered on-device from an hn table with
             one indirect DMA per 128 edges (slow: ~1.3us/call Pool-bound,
             kept as the fully-on-device reference implementation).
"""
import math
import numpy as np

N_NODES = 100000
N_EDGES = 1600000
DIM = 32
N_CORES = 8
BN_EPS = 1e-5
PAD_E = -40.0
P = 128

GATHER_MODE = "halo"

CLASSES = [2, 4, 6, 8, 10, 12, 14, 16, 18, 20, 22, 24, 26, 28, 30, 32,
           36, 40, 44, 48, 56, 64, 96, 128]
MAX_TILE_COLS = 1536


class Cfg:
    def __init__(self, n_nodes=N_NODES, n_edges=N_EDGES, n_cores=N_CORES,
                 max_tile_cols=MAX_TILE_COLS, gather_mode=None):
        assert n_nodes % n_cores == 0
        self.n_nodes = n_nodes
        self.n_edges = n_edges
        self.n_cores = n_cores
        self.nodes_per_core = n_nodes // n_cores
        self.n_pad = ((n_nodes + 2 * P - 1) // (2 * P)) * (2 * P)
        self.tab_cols = self.n_pad // P
        self.max_tile_cols = max_tile_cols
        self.gather_mode = gather_mode or GATHER_MODE


def _lcm(a, b):
    return a * b // math.gcd(a, b)


# ---------------------------------------------------------------------------
# host-side routing / marshalling
# ---------------------------------------------------------------------------

def host_prep(h, e, norm, src, dst, cfg):
    n, nc_, npc = cfg.n_nodes, cfg.n_cores, cfg.nodes_per_core
    h = np.asarray(h, np.float32)
    e = np.asarray(e, np.float32)
    norm = np.asarray(norm, np.float32).reshape(-1)
    src = np.asarray(src, np.int64)
    dst = np.asarray(dst, np.int64)

    deg = np.bincount(dst, minlength=n)
    cls_arr = np.array(CLASSES, np.int64)
    w_node = cls_arr[np.searchsorted(cls_arr, np.maximum(deg, 1))]
    assert deg.max() <= CLASSES[-1]

    node_core = np.arange(n) // npc
    budgets = {}
    for w in CLASSES:
        sel = w_node == w
        if not sel.any():
            continue
        cnt = np.bincount(node_core[sel], minlength=nc_).max()
        m = 4 * (P // math.gcd(w, P))
        budgets[w] = int(((cnt + m - 1) // m) * m)

    class_list = [w for w in CLASSES if w in budgets]
    blk_nodes = {w: budgets[w] // 4 for w in class_list}
    node_off, col_off = {}, {}
    no = co = 0
    for w in class_list:
        node_off[w], col_off[w] = no, co
        no += blk_nodes[w]
        co += blk_nodes[w] * w
    nslot, totc = no, co

    tile_plan = []
    for w in class_list:
        cols = blk_nodes[w] * w
        step = _lcm(w, P)
        cmax = max((cfg.max_tile_cols // step) * step, step)
        off = 0
        while off < cols:
            c = min(cmax, cols - off)
            tile_plan.append((w, node_off[w] + off // w, col_off[w] + off, c))
            off += c
    tile_starts = np.array([t[2] for t in tile_plan], np.int64)
    n_tiles = len(tile_plan)
    slots = 4 * totc
    S_total = slots // P

    noff_lut = np.zeros(CLASSES[-1] + 1, np.int64)
    coff_lut = np.zeros(CLASSES[-1] + 1, np.int64)
    for w in class_list:
        noff_lut[w] = node_off[w]
        coff_lut[w] = col_off[w]

    mtc = max(t[3] for t in tile_plan)
    plan = dict(cfg=cfg, class_list=class_list, budgets=budgets,
                blk_nodes=blk_nodes, node_off=node_off, col_off=col_off,
                nslot=nslot, totc=totc, tile_plan=tile_plan, n_tiles=n_tiles,
                mtc=mtc)

    h_marsh = np.ascontiguousarray(
        np.pad(h, ((0, cfg.n_pad - n), (0, 0))).reshape(P, cfg.tab_cols * DIM))
    norm_marsh = np.ascontiguousarray(
        np.pad(norm, (0, cfg.n_pad - n)).reshape(P, cfg.tab_cols))
    foldw = np.zeros((P, DIM), np.float32)
    foldw[np.arange(P), np.arange(P) % DIM] = 1.0
    brep = np.zeros((4, P), np.float32)
    for a in range(4):
        brep[a, 32 * a:32 * a + 32] = 1.0

    core_e = dst // npc
    order = np.lexsort((dst, core_e))

    in_maps, reasm = [], []
    for k in range(nc_):
        lo = k * npc
        o_k = order[core_e[order] == k]
        k_src, k_dst = src[o_k], dst[o_k]
        nd_loc = k_dst - lo
        w_e = w_node[k_dst]

        w_loc = w_node[lo:lo + npc]
        node_col = np.zeros(npc, np.int64)
        node_blk = np.zeros(npc, np.int64)
        hco = np.zeros((P, nslot), np.float32)
        normco = np.zeros((P, nslot), np.float32)
        for w in class_list:
            nodes_w = np.nonzero(w_loc == w)[0]
            bn = blk_nodes[w]
            i = np.arange(len(nodes_w))
            blk = i // bn
            c = node_off[w] + (i % bn)
            node_blk[nodes_w] = blk
            node_col[nodes_w] = c
            for a in range(4):
                m = blk == a
                if m.any():
                    g = nodes_w[m] + lo
                    hco[32 * a:32 * a + 32, c[m]] = h[g].T
                    normco[32 * a:32 * a + 32, c[m]] = norm[g][None, :]

        # per-edge rank within node group (edges sorted by dst)
        chg = np.ones(len(nd_loc), bool)
        chg[1:] = nd_loc[1:] != nd_loc[:-1]
        starts = np.nonzero(chg)[0]
        rank = np.arange(len(nd_loc)) - np.repeat(
            starts, np.diff(np.append(starts, len(nd_loc))))

        blk_e = node_blk[nd_loc]
        gc = coff_lut[w_e] + (node_col[nd_loc] - noff_lut[w_e]) * w_e + rank
        t_idx = np.searchsorted(tile_starts, gc, side="right") - 1
        cc = gc - tile_starts[t_idx]
        g_slot = (4 * (cc // P) + blk_e) * P + (cc % P)   # gather position

        e_fm = np.full((P, totc), PAD_E, np.float32)
        rmask = np.zeros((P, totc), np.uint8)
        feat = np.arange(DIM)
        rows = (32 * blk_e[:, None] + feat[None, :]).ravel()
        colz = np.repeat(gc, DIM)
        e_fm[rows, colz] = e[o_k].ravel()
        rmask[rows, colz] = 1

        im = {
            "norm_marsh": norm_marsh, "e_fm": e_fm, "rmask": rmask,
            "hco": hco, "normco": normco, "foldw": foldw,
            "consts": np.zeros((DIM, 8), np.float32),
        }
        if cfg.gather_mode == "device":
            gidx = np.zeros((P, S_total), np.int32)
            gidx[g_slot % P, g_slot // P] = k_src
            im["gidx"] = gidx
            im["h_marsh"] = h_marsh
        else:
            hsrc_fm = np.zeros((P, totc), np.float32)
            hsrc_fm[rows, colz] = h[k_src].ravel()
            norm4 = np.zeros((4, totc), np.float32)
            norm4[blk_e, gc] = norm[k_src]
            im["hsrc_fm"] = hsrc_fm
            im["norm4"] = norm4
            im["brep"] = brep
        in_maps.append(im)
        reasm.append(dict(lo=lo, node_col=node_col, node_blk=node_blk,
                          eid=o_k, blk=blk_e, gcol=gc))
    return plan, in_maps, reasm


def set_bn_params(in_maps, gamma_h, beta_h, gamma_e, beta_e):
    for im in in_maps:
        c = im["consts"]
        c[:, 0], c[:, 1], c[:, 2], c[:, 3] = gamma_h, beta_h, gamma_e, beta_e


# ---------------------------------------------------------------------------
# device program
# ---------------------------------------------------------------------------

def build_program(plan):
    import concourse.bass as bass
    import concourse.bacc as bacc
    import concourse.tile as tile
    from concourse import mybir
    from concourse.masks import make_identity

    cfg = plan["cfg"]
    nslot, totc = plan["nslot"], plan["totc"]
    tile_plan, n_tiles = plan["tile_plan"], plan["n_tiles"]
    slots = 4 * totc
    S_total = slots // P
    MTC = plan["mtc"]
    device_gather = cfg.gather_mode == "device"
    f32, i32, u8 = mybir.dt.float32, mybir.dt.int32, mybir.dt.uint8
    AF = mybir.ActivationFunctionType
    OP = mybir.AluOpType
    AX = mybir.AxisListType

    nc = bacc.Bacc("TRN2", target_bir_lowering=False, debug=False,
                   num_devices=cfg.n_cores)

    def din(name, shape, dt=f32):
        return nc.dram_tensor(name, shape, dt, kind="ExternalInput").ap()

    norm_marsh = din("norm_marsh", [P, cfg.tab_cols])
    e_fm = din("e_fm", [P, totc])
    rmask_d = din("rmask", [P, totc], u8)
    hco_d = din("hco", [P, nslot])
    normco_d = din("normco", [P, nslot])
    foldw_d = din("foldw", [P, DIM])
    consts_d = din("consts", [DIM, 8])
    if device_gather:
        h_marsh = din("h_marsh", [P, cfg.tab_cols * DIM])
        gidx = din("gidx", [P, S_total], i32)
    else:
        hsrc_d = din("hsrc_fm", [P, totc])
        norm4_d = din("norm4", [4, totc])
        brep_d = din("brep", [4, P])

    e_out_T = nc.dram_tensor("e_out_T", [P, totc], f32, kind="ExternalOutput").ap()
    h_out_T = nc.dram_tensor("h_out_T", [P, nslot], f32, kind="ExternalOutput").ap()

    with tile.TileContext(nc) as tc:
        with tc.tile_pool(name="pers", bufs=1) as pers, \
             tc.tile_pool(name="sb", bufs=2) as sb, \
             tc.tile_pool(name="gpool", bufs=2) as gpool, \
             tc.tile_pool(name="psum", bufs=1, space="PSUM") as psum, \
             tc.tile_pool(name="psum1", bufs=1, space="PSUM") as psum1, \
             tc.tile_pool(name="dram", bufs=1, space="DRAM") as dram:

            eij_st = dram.tile([P, totc], f32)

            if device_gather:
                hn_tab = dram.tile([cfg.n_pad, DIM], f32)
                ident = pers.tile([P, P], f32)
                make_identity(nc, ident[:])
                # hn table: hn = h * norm
                TB = 64
                hn_tab_pm = hn_tab[:].rearrange("(p c) f -> p (c f)", p=P)
                for c0 in range(0, cfg.tab_cols, TB):
                    c1 = min(c0 + TB, cfg.tab_cols)
                    w = c1 - c0
                    ht = sb.tile([P, TB * DIM], f32, tag="ht")
                    nt = sb.tile([P, TB], f32, tag="nt")
                    nc.sync.dma_start(out=ht[:, :w * DIM],
                                      in_=h_marsh[:, c0 * DIM:c1 * DIM])
                    nc.sync.dma_start(out=nt[:, :w], in_=norm_marsh[:, c0:c1])
                    nc.vector.tensor_tensor(
                        out=ht[:, :w * DIM].rearrange("p (c f) -> p c f", f=DIM),
                        in0=ht[:, :w * DIM].rearrange("p (c f) -> p c f", f=DIM),
                        in1=nt[:, :w, None].broadcast_to([P, w, DIM]),
                        op=OP.mult)
                    nc.sync.dma_start(out=hn_tab_pm[:, c0 * DIM:c1 * DIM],
                                      in_=ht[:, :w * DIM])
            else:
                brep_sb = pers.tile([4, P], f32)
                nc.sync.dma_start(out=brep_sb[:], in_=brep_d[:])

            hnco = pers.tile([P, nslot], f32)
            nco = pers.tile([P, nslot], f32)
            nc.sync.dma_start(out=hnco[:], in_=hco_d[:])
            nc.sync.dma_start(out=nco[:], in_=normco_d[:])
            nc.vector.tensor_tensor(out=hnco[:], in0=hnco[:], in1=nco[:], op=OP.mult)

            num0 = pers.tile([P, nslot], f32)
            den0 = pers.tile([P, nslot], f32)
            den1 = pers.tile([P, nslot], f32)
            esum_acc = pers.tile([P, n_tiles], f32)
            esq_acc = pers.tile([P, n_tiles], f32)

            # ================= pass 1 =================
            for ti, (w, nd_off, c_off, C) in enumerate(tile_plan):
                S = C // 32
                nn = C // w

                et = sb.tile([P, MTC], f32, tag="et")
                nc.sync.dma_start(out=et[:, :C], in_=e_fm[:, c_off:c_off + C])
                rm = sb.tile([P, MTC], u8, tag="rm")
                nc.sync.dma_start(out=rm[:, :C], in_=rmask_d[:, c_off:c_off + C])

                if device_gather:
                    ix = sb.tile([P, MTC // 32], i32, tag="ix")
                    nc.sync.dma_start(
                        out=ix[:, :S], in_=gidx[:, 4 * c_off // P:
                                                (4 * c_off + 4 * C) // P])
                    g = gpool.tile([P, MTC], f32, tag="g")
                    for s in range(S):
                        nc.gpsimd.indirect_dma_start(
                            out=g[:, s * 32:(s + 1) * 32],
                            out_offset=None,
                            in_=hn_tab[:],
                            in_offset=bass.IndirectOffsetOnAxis(
                                ap=ix[:, s:s + 1], axis=0),
                        )
                    hT = psum.tile([P, MTC], f32, tag="hT", space="PSUM")
                    for j in range(C // P):
                        nc.tensor.transpose(out=hT[:, j * P:(j + 1) * P],
                                            in_=g[:, j * P:(j + 1) * P],
                                            identity=ident[:])
                else:
                    n4 = sb.tile([4, MTC], f32, tag="n4")
                    nc.sync.dma_start(out=n4[:, :C], in_=norm4_d[:, c_off:c_off + C])
                    hs = sb.tile([P, MTC], f32, tag="hs")
                    nc.sync.dma_start(out=hs[:, :C], in_=hsrc_d[:, c_off:c_off + C])
                    nrm = psum.tile([P, MTC], f32, tag="hT", space="PSUM")
                    for q0 in range(0, C, 512):
                        q1 = min(q0 + 512, C)
                        nc.tensor.matmul(out=nrm[:, q0:q1], lhsT=brep_sb[:],
                                         rhs=n4[:, q0:q1], start=True, stop=True)
                    hT = gpool.tile([P, MTC], f32, tag="g")
                    nc.vector.tensor_tensor(out=hT[:, :C], in0=hs[:, :C],
                                            in1=nrm[:, :C], op=OP.mult)

                s_t = sb.tile([P, MTC], f32, tag="s_t")
                nc.vector.tensor_tensor(
                    out=s_t[:, :C].rearrange("p (n w) -> p n w", w=w),
                    in0=hT[:, :C].rearrange("p (n w) -> p n w", w=w),
                    in1=hnco[:, nd_off:nd_off + nn, None].broadcast_to([P, nn, w]),
                    op=OP.add)
                eij = sb.tile([P, MTC], f32, tag="eij")
                nc.gpsimd.tensor_tensor(out=eij[:, :C], in0=et[:, :C],
                                        in1=s_t[:, :C], op=OP.add)
                nc.sync.dma_start(out=eij_st[:, c_off:c_off + C], in_=eij[:, :C])

                sig = sb.tile([P, MTC], f32, tag="sig")
                nc.scalar.activation(out=sig[:, :C], in_=eij[:, :C], func=AF.Sigmoid)
                em = sb.tile([P, MTC], f32, tag="em")
                nc.vector.tensor_tensor(out=em[:, :C], in0=eij[:, :C],
                                        in1=rm[:, :C], op=OP.mult)
                nc.scalar.activation(out=s_t[:, :C], in_=em[:, :C], func=AF.Copy,
                                     accum_out=esum_acc[:, ti:ti + 1])
                nc.scalar.activation(out=s_t[:, :C], in_=em[:, :C], func=AF.Square,
                                     accum_out=esq_acc[:, ti:ti + 1])

                # x = sigma * h_src (reuse em)
                nc.vector.tensor_tensor(out=em[:, :C], in0=sig[:, :C],
                                        in1=hT[:, :C], op=OP.mult)
                nc.vector.tensor_reduce(
                    out=num0[:, nd_off:nd_off + nn],
                    in_=em[:, :C].rearrange("p (n w) -> p n w", w=w),
                    axis=AX.X, op=OP.add)
                nc.vector.tensor_reduce(
                    out=den0[:, nd_off:nd_off + nn],
                    in_=sig[:, :C].rearrange("p (n w) -> p n w", w=w),
                    axis=AX.X, op=OP.add)

            # ================= node pipeline =================
            nc.vector.tensor_scalar_add(den0[:], den0[:], 1e-6)
            nc.vector.reciprocal(out=den1[:], in_=den0[:])
            nc.vector.tensor_tensor(out=num0[:], in0=num0[:], in1=den1[:], op=OP.mult)
            nc.vector.tensor_tensor(out=num0[:], in0=num0[:], in1=hnco[:], op=OP.add)
            nc.vector.tensor_tensor(out=num0[:], in0=num0[:], in1=nco[:], op=OP.mult)
            hnew = num0

            stats4 = pers.tile([P, 4], f32)
            nc.vector.tensor_reduce(out=stats4[:, 0:1], in_=esum_acc[:],
                                    axis=AX.X, op=OP.add)
            nc.vector.tensor_reduce(out=stats4[:, 1:2], in_=esq_acc[:],
                                    axis=AX.X, op=OP.add)
            nc.scalar.activation(out=den0[:], in_=hnew[:], func=AF.Copy,
                                 accum_out=stats4[:, 2:3])
            nc.scalar.activation(out=den0[:], in_=hnew[:], func=AF.Square,
                                 accum_out=stats4[:, 3:4])

            foldw_sb = pers.tile([P, DIM], f32)
            nc.sync.dma_start(out=foldw_sb[:], in_=foldw_d[:])
            fold_ps = psum1.tile([DIM, 4], f32, space="PSUM")
            nc.tensor.matmul(out=fold_ps[:], lhsT=foldw_sb[:], rhs=stats4[:],
                             start=True, stop=True)
            consts_sb = pers.tile([DIM, 8], f32)
            nc.sync.dma_start(out=consts_sb[:], in_=consts_d[:])
            fold_sb = pers.tile([DIM, 4], f32)
            nc.vector.tensor_copy(out=fold_sb[:], in_=fold_ps[:])

            cc_in = dram.tile([DIM, 4], f32)
            cc_out = dram.tile([DIM, 4], f32)
            nc.sync.dma_start(out=cc_in[:], in_=fold_sb[:])
            nc.gpsimd.collective_compute(
                "AllReduce", OP.add,
                replica_groups=[list(range(cfg.n_cores))],
                ins=[cc_in[:].opt()], outs=[cc_out[:].opt()])
            gstats = pers.tile([DIM, 4], f32)
            nc.sync.dma_start(out=gstats[:], in_=cc_out[:])

            coef = pers.tile([DIM, 4], f32)
            tmp = pers.tile([DIM, 4], f32)
            nc.vector.tensor_scalar_mul(tmp[:, 0:1], gstats[:, 0:1], 1.0 / cfg.n_edges)
            nc.vector.tensor_scalar_mul(tmp[:, 1:2], gstats[:, 1:2], 1.0 / cfg.n_edges)
            nc.vector.tensor_scalar_mul(tmp[:, 2:3], gstats[:, 2:3], 1.0 / cfg.n_nodes)
            nc.vector.tensor_scalar_mul(tmp[:, 3:4], gstats[:, 3:4], 1.0 / cfg.n_nodes)
            var2 = pers.tile([DIM, 2], f32)
            nc.vector.tensor_tensor(out=var2[:, 0:1], in0=tmp[:, 0:1],
                                    in1=tmp[:, 0:1], op=OP.mult)
            nc.vector.tensor_tensor(out=var2[:, 1:2], in0=tmp[:, 2:3],
                                    in1=tmp[:, 2:3], op=OP.mult)
            nc.vector.tensor_tensor(out=var2[:, 0:1], in0=tmp[:, 1:2],
                                    in1=var2[:, 0:1], op=OP.subtract)
            nc.vector.tensor_tensor(out=var2[:, 1:2], in0=tmp[:, 3:4],
                                    in1=var2[:, 1:2], op=OP.subtract)
            nc.vector.tensor_scalar_add(var2[:], var2[:], BN_EPS)
            std2 = pers.tile([DIM, 2], f32)
            nc.scalar.sqrt(out=std2[:], in_=var2[:])
            rstd2 = pers.tile([DIM, 2], f32)
            nc.vector.reciprocal(out=rstd2[:], in_=std2[:])
            nc.vector.tensor_tensor(out=coef[:, 0:1], in0=consts_sb[:, 2:3],
                                    in1=rstd2[:, 0:1], op=OP.mult)
            nc.vector.tensor_tensor(out=coef[:, 2:3], in0=consts_sb[:, 0:1],
                                    in1=rstd2[:, 1:2], op=OP.mult)
            mba = pers.tile([DIM, 2], f32)
            nc.vector.tensor_tensor(out=mba[:, 0:1], in0=tmp[:, 0:1],
                                    in1=coef[:, 0:1], op=OP.mult)
            nc.vector.tensor_tensor(out=mba[:, 1:2], in0=tmp[:, 2:3],
                                    in1=coef[:, 2:3], op=OP.mult)
            nc.vector.tensor_tensor(out=coef[:, 1:2], in0=consts_sb[:, 3:4],
                                    in1=mba[:, 0:1], op=OP.subtract)
            nc.vector.tensor_tensor(out=coef[:, 3:4], in0=consts_sb[:, 1:2],
                                    in1=mba[:, 1:2], op=OP.subtract)

            coef_dr = dram.tile([DIM, 4], f32)
            nc.sync.dma_start(out=coef_dr[:], in_=coef[:])
            crep = pers.tile([P, 4], f32)
            for b in range(4):
                nc.sync.dma_start(out=crep[32 * b:32 * b + 32, :], in_=coef_dr[:])

            nc.scalar.activation(out=den0[:], in_=hnew[:], func=AF.Relu,
                                 scale=crep[:, 2:3], bias=crep[:, 3:4])
            nc.sync.dma_start(out=h_out_T[:], in_=den0[:])

            # ================= pass 2: e_out =================
            for (w, nd_off, c_off, C) in tile_plan:
                e2 = sb.tile([P, MTC], f32, tag="et")
                nc.sync.dma_start(out=e2[:, :C], in_=eij_st[:, c_off:c_off + C])
                nc.scalar.activation(out=e2[:, :C], in_=e2[:, :C], func=AF.Relu,
                                     scale=crep[:, 0:1], bias=crep[:, 1:2])
                nc.sync.dma_start(out=e_out_T[:, c_off:c_off + C], in_=e2[:, :C])

    nc.compile()
    return nc


# ---------------------------------------------------------------------------
# reassembly
# ---------------------------------------------------------------------------

def reassemble(plan, reasm, results, cfg):
    h_out = np.zeros((cfg.n_nodes, DIM), np.float32)
    e_out = np.zeros((cfg.n_edges, DIM), np.float32)
    feat = np.arange(DIM)
    npc = cfg.nodes_per_core
    for k, r in enumerate(reasm):
        eT = np.asarray(results[k]["e_out_T"])
        hT = np.asarray(results[k]["h_out_T"])
        rows = 32 * r["node_blk"][:, None] + feat[None, :]
        h_out[r["lo"]:r["lo"] + npc] = hT[rows, r["node_col"][:, None]]
        rows = 32 * r["blk"][:, None] + feat[None, :]
        e_out[r["eid"]] = eT[rows, r["gcol"][:, None]]
    return h_out, e_out


# ---------------------------------------------------------------------------
# entry point
# ---------------------------------------------------------------------------

_CACHE = {}


def kernel(h, e, norm, src, dst, gamma_h, beta_h, gamma_e, beta_e,
           _cfg=None, _sim=False):
    cfg = _cfg or Cfg()
    plan, in_maps, reasm = host_prep(h, e, norm, src, dst, cfg)
    set_bn_params(in_maps,
                  np.asarray(gamma_h, np.float32), np.asarray(beta_h, np.float32),
                  np.asarray(gamma_e, np.float32), np.asarray(beta_e, np.float32))

    sig = (cfg.n_nodes, cfg.n_edges, cfg.gather_mode,
           tuple(sorted(plan["budgets"].items())))
    if sig in _CACHE:
        nc = _CACHE[sig]
    else:
        nc = build_program(plan)
        _CACHE[sig] = nc

    if _sim:
        import concourse.bass_interp as bass_interp
        sim = bass_interp.MultiCoreSim(nc, cfg.n_cores)
        for k in range(cfg.n_cores):
            for name, val in in_maps[k].items():
                sim.cores[k].tensor(name)[:] = val
        sim.simulate()
        results = [{nm: np.array(sim.cores[k].mem_tensor(nm))
                    for nm in ("e_out_T", "h_out_T")} for k in range(cfg.n_cores)]
    else:
        from concourse.bass_utils import run_bass_kernel_spmd
        res = run_bass_kernel_spmd(nc, in_maps, list(range(cfg.n_cores)))
        results = res.results

    return reassemble(plan, reasm, results, cfg)


# revision 13
# speedup vs baseline: 1.0456x; 1.0456x over previous
"""GatedGCN layer on 8 Trainium2 NeuronCores (Bass/Tile).

Sharding (per the problem's hint): nodes are partitioned into 8 contiguous
ranges; each core owns the edges whose dst is in its range, so the segment-sum
scatter is core-local. The host routes each edge to its dst core together with
its payload (e row, src id, and - in halo mode - the boundary src node's raw
h/norm features, i.e. the materialized "all-gather of boundary src features"
from the hint). Edges are dst-sorted and padded so each node occupies a fixed
W-slot group (W = degree class); segment sums then become fixed-window Vector
engine reductions in a feature-major layout [128 = 4 blocks x 32 features,
columns]. h_dst is free via stride-0 broadcast APs over a resident per-core
node table. All arithmetic of the reference (h*norm - including for the
halo-replicated src copies - gate, sigmoid, weighted mean, batch-norm, relu)
runs on device. BatchNorm statistics ride the Scalar engine's accumulator
(pad slots excluded exactly via a 0/1 mask), are folded by a tiny PE matmul,
AllReduce'd across the 8 cores, and applied as one Relu-affine activation per
tile. e_ij is staged to DRAM in f32 between the stats pass and the output
pass.

GATHER_MODE:
  "halo"   - src features are routed with the edges by the host (fast path).
  "device" - src features are gathered on-device from an hn table with
             one indirect DMA per 128 edges (slow: ~1.3us/call Pool-bound,
             kept as the fully-on-device reference implementation).
"""
import math
import numpy as np

N_NODES = 100000
N_EDGES = 1600000
DIM = 32
N_CORES = 8
BN_EPS = 1e-5
PAD_E = -40.0
P = 128

GATHER_MODE = "halo"

CLASSES = [2, 4, 6, 8, 10, 12, 14, 16, 18, 20, 22, 24, 28, 32,
           36, 40, 48, 56, 64, 96, 128]
MAX_TILE_COLS = 1536


class Cfg:
    def __init__(self, n_nodes=N_NODES, n_edges=N_EDGES, n_cores=N_CORES,
                 max_tile_cols=MAX_TILE_COLS, gather_mode=None):
        assert n_nodes % n_cores == 0
        self.n_nodes = n_nodes
        self.n_edges = n_edges
        self.n_cores = n_cores
        self.nodes_per_core = n_nodes // n_cores
        self.n_pad = ((n_nodes + 2 * P - 1) // (2 * P)) * (2 * P)
        self.tab_cols = self.n_pad // P
        self.max_tile_cols = max_tile_cols
        self.gather_mode = gather_mode or GATHER_MODE


def _lcm(a, b):
    return a * b // math.gcd(a, b)


# ---------------------------------------------------------------------------
# host-side routing / marshalling
# ---------------------------------------------------------------------------

def host_prep(h, e, norm, src, dst, cfg):
    n, nc_, npc = cfg.n_nodes, cfg.n_cores, cfg.nodes_per_core
    h = np.asarray(h, np.float32)
    e = np.asarray(e, np.float32)
    norm = np.asarray(norm, np.float32).reshape(-1)
    src = np.asarray(src, np.int64)
    dst = np.asarray(dst, np.int64)

    deg = np.bincount(dst, minlength=n)
    cls_arr = np.array(CLASSES, np.int64)
    w_node = cls_arr[np.searchsorted(cls_arr, np.maximum(deg, 1))]
    assert deg.max() <= CLASSES[-1]

    node_core = np.arange(n) // npc
    budgets = {}
    for w in CLASSES:
        sel = w_node == w
        if not sel.any():
            continue
        cnt = np.bincount(node_core[sel], minlength=nc_).max()
        m = 4 * (P // math.gcd(w, P))
        budgets[w] = int(((cnt + m - 1) // m) * m)

    class_list = [w for w in CLASSES if w in budgets]
    blk_nodes = {w: budgets[w] // 4 for w in class_list}
    node_off, col_off = {}, {}
    no = co = 0
    for w in class_list:
        node_off[w], col_off[w] = no, co
        no += blk_nodes[w]
        co += blk_nodes[w] * w
    nslot, totc = no, co

    tile_plan = []
    for w in class_list:
        cols = blk_nodes[w] * w
        step = _lcm(w, P)
        cmax = max((cfg.max_tile_cols // step) * step, step)
        off = 0
        while off < cols:
            c = min(cmax, cols - off)
            tile_plan.append((w, node_off[w] + off // w, col_off[w] + off, c))
            off += c
    tile_starts = np.array([t[2] for t in tile_plan], np.int64)
    n_tiles = len(tile_plan)
    slots = 4 * totc
    S_total = slots // P

    noff_lut = np.zeros(CLASSES[-1] + 1, np.int64)
    coff_lut = np.zeros(CLASSES[-1] + 1, np.int64)
    for w in class_list:
        noff_lut[w] = node_off[w]
        coff_lut[w] = col_off[w]

    mtc = max(t[3] for t in tile_plan)
    plan = dict(cfg=cfg, class_list=class_list, budgets=budgets,
                blk_nodes=blk_nodes, node_off=node_off, col_off=col_off,
                nslot=nslot, totc=totc, tile_plan=tile_plan, n_tiles=n_tiles,
                mtc=mtc)

    h_marsh = np.ascontiguousarray(
        np.pad(h, ((0, cfg.n_pad - n), (0, 0))).reshape(P, cfg.tab_cols * DIM))
    norm_marsh = np.ascontiguousarray(
        np.pad(norm, (0, cfg.n_pad - n)).reshape(P, cfg.tab_cols))
    foldw = np.zeros((P, DIM), np.float32)
    foldw[np.arange(P), np.arange(P) % DIM] = 1.0
    brep = np.zeros((4, P), np.float32)
    for a in range(4):
        brep[a, 32 * a:32 * a + 32] = 1.0

    core_e = dst // npc
    order = np.lexsort((dst, core_e))

    in_maps, reasm = [], []
    for k in range(nc_):
        lo = k * npc
        o_k = order[core_e[order] == k]
        k_src, k_dst = src[o_k], dst[o_k]
        nd_loc = k_dst - lo
        w_e = w_node[k_dst]

        w_loc = w_node[lo:lo + npc]
        node_col = np.zeros(npc, np.int64)
        node_blk = np.zeros(npc, np.int64)
        hco = np.zeros((P, nslot), np.float32)
        normco = np.zeros((P, nslot), np.float32)
        for w in class_list:
            nodes_w = np.nonzero(w_loc == w)[0]
            bn = blk_nodes[w]
            i = np.arange(len(nodes_w))
            blk = i // bn
            c = node_off[w] + (i % bn)
            node_blk[nodes_w] = blk
            node_col[nodes_w] = c
            for a in range(4):
                m = blk == a
                if m.any():
                    g = nodes_w[m] + lo
                    hco[32 * a:32 * a + 32, c[m]] = h[g].T
                    normco[32 * a:32 * a + 32, c[m]] = norm[g][None, :]

        # per-edge rank within node group (edges sorted by dst)
        chg = np.ones(len(nd_loc), bool)
        chg[1:] = nd_loc[1:] != nd_loc[:-1]
        starts = np.nonzero(chg)[0]
        rank = np.arange(len(nd_loc)) - np.repeat(
            starts, np.diff(np.append(starts, len(nd_loc))))

        blk_e = node_blk[nd_loc]
        gc = coff_lut[w_e] + (node_col[nd_loc] - noff_lut[w_e]) * w_e + rank
        t_idx = np.searchsorted(tile_starts, gc, side="right") - 1
        cc = gc - tile_starts[t_idx]
        g_slot = (4 * (cc // P) + blk_e) * P + (cc % P)   # gather position

        e_fm = np.full((P, totc), PAD_E, np.float32)
        rmask = np.zeros((P, totc), np.uint8)
        feat = np.arange(DIM)
        rows = (32 * blk_e[:, None] + feat[None, :]).ravel()
        colz = np.repeat(gc, DIM)
        e_fm[rows, colz] = e[o_k].ravel()
        rmask[rows, colz] = 1

        im = {
            "norm_marsh": norm_marsh, "e_fm": e_fm, "rmask": rmask,
            "hco": hco, "normco": normco, "foldw": foldw,
            "consts": np.zeros((DIM, 8), np.float32),
        }
        if cfg.gather_mode == "device":
            gidx = np.zeros((P, S_total), np.int32)
            gidx[g_slot % P, g_slot // P] = k_src
            im["gidx"] = gidx
            im["h_marsh"] = h_marsh
        else:
            hsrc_fm = np.zeros((P, totc), np.float32)
            hsrc_fm[rows, colz] = h[k_src].ravel()
            norm4 = np.zeros((4, totc), np.float32)
            norm4[blk_e, gc] = norm[k_src]
            im["hsrc_fm"] = hsrc_fm
            im["norm4"] = norm4
            im["brep"] = brep
        in_maps.append(im)
        reasm.append(dict(lo=lo, node_col=node_col, node_blk=node_blk,
                          eid=o_k, blk=blk_e, gcol=gc))
    return plan, in_maps, reasm


def set_bn_params(in_maps, gamma_h, beta_h, gamma_e, beta_e):
    for im in in_maps:
        c = im["consts"]
        c[:, 0], c[:, 1], c[:, 2], c[:, 3] = gamma_h, beta_h, gamma_e, beta_e


# ---------------------------------------------------------------------------
# device program
# ---------------------------------------------------------------------------

def build_program(plan):
    import concourse.bass as bass
    import concourse.bacc as bacc
    import concourse.tile as tile
    from concourse import mybir
    from concourse.masks import make_identity

    cfg = plan["cfg"]
    nslot, totc = plan["nslot"], plan["totc"]
    tile_plan, n_tiles = plan["tile_plan"], plan["n_tiles"]
    slots = 4 * totc
    S_total = slots // P
    MTC = plan["mtc"]
    device_gather = cfg.gather_mode == "device"
    f32, i32, u8 = mybir.dt.float32, mybir.dt.int32, mybir.dt.uint8
    AF = mybir.ActivationFunctionType
    OP = mybir.AluOpType
    AX = mybir.AxisListType

    nc = bacc.Bacc("TRN2", target_bir_lowering=False, debug=False,
                   num_devices=cfg.n_cores)

    def din(name, shape, dt=f32):
        return nc.dram_tensor(name, shape, dt, kind="ExternalInput").ap()

    norm_marsh = din("norm_marsh", [P, cfg.tab_cols])
    e_fm = din("e_fm", [P, totc])
    rmask_d = din("rmask", [P, totc], u8)
    hco_d = din("hco", [P, nslot])
    normco_d = din("normco", [P, nslot])
    foldw_d = din("foldw", [P, DIM])
    consts_d = din("consts", [DIM, 8])
    if device_gather:
        h_marsh = din("h_marsh", [P, cfg.tab_cols * DIM])
        gidx = din("gidx", [P, S_total], i32)
    else:
        hsrc_d = din("hsrc_fm", [P, totc])
        norm4_d = din("norm4", [4, totc])
        brep_d = din("brep", [4, P])

    e_out_T = nc.dram_tensor("e_out_T", [P, totc], f32, kind="ExternalOutput").ap()
    h_out_T = nc.dram_tensor("h_out_T", [P, nslot], f32, kind="ExternalOutput").ap()

    with tile.TileContext(nc) as tc:
        with tc.tile_pool(name="pers", bufs=1) as pers, \
             tc.tile_pool(name="sb", bufs=2) as sb, \
             tc.tile_pool(name="gpool", bufs=2) as gpool, \
             tc.tile_pool(name="psum", bufs=2, space="PSUM") as psum, \
             tc.tile_pool(name="psum1", bufs=1, space="PSUM") as psum1, \
             tc.tile_pool(name="dram", bufs=1, space="DRAM") as dram:

            eij_st = dram.tile([P, totc], f32)

            if device_gather:
                hn_tab = dram.tile([cfg.n_pad, DIM], f32)
                ident = pers.tile([P, P], f32)
                make_identity(nc, ident[:])
                # hn table: hn = h * norm
                TB = 64
                hn_tab_pm = hn_tab[:].rearrange("(p c) f -> p (c f)", p=P)
                for c0 in range(0, cfg.tab_cols, TB):
                    c1 = min(c0 + TB, cfg.tab_cols)
                    w = c1 - c0
                    ht = sb.tile([P, TB * DIM], f32, tag="ht")
                    nt = sb.tile([P, TB], f32, tag="nt")
                    nc.sync.dma_start(out=ht[:, :w * DIM],
                                      in_=h_marsh[:, c0 * DIM:c1 * DIM])
                    nc.sync.dma_start(out=nt[:, :w], in_=norm_marsh[:, c0:c1])
                    nc.vector.tensor_tensor(
                        out=ht[:, :w * DIM].rearrange("p (c f) -> p c f", f=DIM),
                        in0=ht[:, :w * DIM].rearrange("p (c f) -> p c f", f=DIM),
                        in1=nt[:, :w, None].broadcast_to([P, w, DIM]),
                        op=OP.mult)
                    nc.sync.dma_start(out=hn_tab_pm[:, c0 * DIM:c1 * DIM],
                                      in_=ht[:, :w * DIM])
            else:
                brep_sb = pers.tile([4, P], f32)
                nc.sync.dma_start(out=brep_sb[:], in_=brep_d[:])

            hnco = pers.tile([P, nslot], f32)
            nco = pers.tile([P, nslot], f32)
            nc.sync.dma_start(out=hnco[:], in_=hco_d[:])
            nc.sync.dma_start(out=nco[:], in_=normco_d[:])
            nc.vector.tensor_tensor(out=hnco[:], in0=hnco[:], in1=nco[:], op=OP.mult)

            num0 = pers.tile([P, nslot], f32)
            den0 = pers.tile([P, nslot], f32)
            den1 = pers.tile([P, nslot], f32)
            esum_acc = pers.tile([P, n_tiles], f32)
            esq_acc = pers.tile([P, n_tiles], f32)

            # ================= pass 1 =================
            for ti, (w, nd_off, c_off, C) in enumerate(tile_plan):
                S = C // 32
                nn = C // w

                et = sb.tile([P, MTC], f32, tag="et")
                nc.sync.dma_start(out=et[:, :C], in_=e_fm[:, c_off:c_off + C])
                rm = sb.tile([P, MTC], u8, tag="rm")
                nc.sync.dma_start(out=rm[:, :C], in_=rmask_d[:, c_off:c_off + C])

                if device_gather:
                    ix = sb.tile([P, MTC // 32], i32, tag="ix")
                    nc.sync.dma_start(
                        out=ix[:, :S], in_=gidx[:, 4 * c_off // P:
                                                (4 * c_off + 4 * C) // P])
                    g = gpool.tile([P, MTC], f32, tag="g")
                    for s in range(S):
                        nc.gpsimd.indirect_dma_start(
                            out=g[:, s * 32:(s + 1) * 32],
                            out_offset=None,
                            in_=hn_tab[:],
                            in_offset=bass.IndirectOffsetOnAxis(
                                ap=ix[:, s:s + 1], axis=0),
                        )
                    hT = psum.tile([P, MTC], f32, tag="hT", space="PSUM")
                    for j in range(C // P):
                        nc.tensor.transpose(out=hT[:, j * P:(j + 1) * P],
                                            in_=g[:, j * P:(j + 1) * P],
                                            identity=ident[:])
                else:
                    n4 = sb.tile([4, MTC], f32, tag="n4")
                    nc.sync.dma_start(out=n4[:, :C], in_=norm4_d[:, c_off:c_off + C])
                    hs = sb.tile([P, MTC], f32, tag="hs")
                    nc.sync.dma_start(out=hs[:, :C], in_=hsrc_d[:, c_off:c_off + C])
                    nrm = psum.tile([P, MTC], f32, tag="hT", space="PSUM")
                    for q0 in range(0, C, 512):
                        q1 = min(q0 + 512, C)
                        nc.tensor.matmul(out=nrm[:, q0:q1], lhsT=brep_sb[:],
                                         rhs=n4[:, q0:q1], start=True, stop=True)
                    hT = gpool.tile([P, MTC], f32, tag="g")
                    nc.vector.tensor_tensor(out=hT[:, :C], in0=hs[:, :C],
                                            in1=nrm[:, :C], op=OP.mult)
                    # (vector reads PSUM here; gpsimd has no PSUM port)

                s_t = sb.tile([P, MTC], f32, tag="s_t")
                nc.vector.tensor_tensor(
                    out=s_t[:, :C].rearrange("p (n w) -> p n w", w=w),
                    in0=hT[:, :C].rearrange("p (n w) -> p n w", w=w),
                    in1=hnco[:, nd_off:nd_off + nn, None].broadcast_to([P, nn, w]),
                    op=OP.add)
                eij = sb.tile([P, MTC], f32, tag="eij")
                nc.gpsimd.tensor_tensor(out=eij[:, :C], in0=et[:, :C],
                                        in1=s_t[:, :C], op=OP.add)
                nc.sync.dma_start(out=eij_st[:, c_off:c_off + C], in_=eij[:, :C])

                sig = sb.tile([P, MTC], f32, tag="sig")
                nc.scalar.activation(out=sig[:, :C], in_=eij[:, :C], func=AF.Sigmoid)
                em = sb.tile([P, MTC], f32, tag="em")
                nc.vector.tensor_tensor(out=em[:, :C], in0=eij[:, :C],
                                        in1=rm[:, :C], op=OP.mult)
                nc.scalar.activation(out=s_t[:, :C], in_=em[:, :C], func=AF.Copy,
                                     accum_out=esum_acc[:, ti:ti + 1])
                nc.scalar.activation(out=s_t[:, :C], in_=em[:, :C], func=AF.Square,
                                     accum_out=esq_acc[:, ti:ti + 1])

                # x = sigma * h_src (reuse em)
                nc.vector.tensor_tensor(out=em[:, :C], in0=sig[:, :C],
                                        in1=hT[:, :C], op=OP.mult)
                nc.vector.tensor_reduce(
                    out=num0[:, nd_off:nd_off + nn],
                    in_=em[:, :C].rearrange("p (n w) -> p n w", w=w),
                    axis=AX.X, op=OP.add)
                nc.vector.tensor_reduce(
                    out=den0[:, nd_off:nd_off + nn],
                    in_=sig[:, :C].rearrange("p (n w) -> p n w", w=w),
                    axis=AX.X, op=OP.add)

            # ================= node pipeline =================
            nc.vector.tensor_scalar_add(den0[:], den0[:], 1e-6)
            nc.vector.reciprocal(out=den1[:], in_=den0[:])
            nc.vector.tensor_tensor(out=num0[:], in0=num0[:], in1=den1[:], op=OP.mult)
            nc.vector.tensor_tensor(out=num0[:], in0=num0[:], in1=hnco[:], op=OP.add)
            nc.vector.tensor_tensor(out=num0[:], in0=num0[:], in1=nco[:], op=OP.mult)
            hnew = num0

            stats4 = pers.tile([P, 4], f32)
            nc.vector.tensor_reduce(out=stats4[:, 0:1], in_=esum_acc[:],
                                    axis=AX.X, op=OP.add)
            nc.vector.tensor_reduce(out=stats4[:, 1:2], in_=esq_acc[:],
                                    axis=AX.X, op=OP.add)
            nc.scalar.activation(out=den0[:], in_=hnew[:], func=AF.Copy,
                                 accum_out=stats4[:, 2:3])
            nc.scalar.activation(out=den0[:], in_=hnew[:], func=AF.Square,
                                 accum_out=stats4[:, 3:4])

            foldw_sb = pers.tile([P, DIM], f32)
            nc.sync.dma_start(out=foldw_sb[:], in_=foldw_d[:])
            fold_ps = psum1.tile([DIM, 4], f32, space="PSUM")
            nc.tensor.matmul(out=fold_ps[:], lhsT=foldw_sb[:], rhs=stats4[:],
                             start=True, stop=True)
            consts_sb = pers.tile([DIM, 8], f32)
            nc.sync.dma_start(out=consts_sb[:], in_=consts_d[:])
            fold_sb = pers.tile([DIM, 4], f32)
            nc.vector.tensor_copy(out=fold_sb[:], in_=fold_ps[:])

            cc_in = dram.tile([DIM, 4], f32)
            cc_out = dram.tile([DIM, 4], f32)
            nc.sync.dma_start(out=cc_in[:], in_=fold_sb[:])
            nc.gpsimd.collective_compute(
                "AllReduce", OP.add,
                replica_groups=[list(range(cfg.n_cores))],
                ins=[cc_in[:].opt()], outs=[cc_out[:].opt()])
            gstats = pers.tile([DIM, 4], f32)
            nc.sync.dma_start(out=gstats[:], in_=cc_out[:])

            coef = pers.tile([DIM, 4], f32)
            tmp = pers.tile([DIM, 4], f32)
            nc.vector.tensor_scalar_mul(tmp[:, 0:1], gstats[:, 0:1], 1.0 / cfg.n_edges)
            nc.vector.tensor_scalar_mul(tmp[:, 1:2], gstats[:, 1:2], 1.0 / cfg.n_edges)
            nc.vector.tensor_scalar_mul(tmp[:, 2:3], gstats[:, 2:3], 1.0 / cfg.n_nodes)
            nc.vector.tensor_scalar_mul(tmp[:, 3:4], gstats[:, 3:4], 1.0 / cfg.n_nodes)
            var2 = pers.tile([DIM, 2], f32)
            nc.vector.tensor_tensor(out=var2[:, 0:1], in0=tmp[:, 0:1],
                                    in1=tmp[:, 0:1], op=OP.mult)
            nc.vector.tensor_tensor(out=var2[:, 1:2], in0=tmp[:, 2:3],
                                    in1=tmp[:, 2:3], op=OP.mult)
            nc.vector.tensor_tensor(out=var2[:, 0:1], in0=tmp[:, 1:2],
                                    in1=var2[:, 0:1], op=OP.subtract)
            nc.vector.tensor_tensor(out=var2[:, 1:2], in0=tmp[:, 3:4],
                                    in1=var2[:, 1:2], op=OP.subtract)
            nc.vector.tensor_scalar_add(var2[:], var2[:], BN_EPS)
            std2 = pers.tile([DIM, 2], f32)
            nc.scalar.sqrt(out=std2[:], in_=var2[:])
            rstd2 = pers.tile([DIM, 2], f32)
            nc.vector.reciprocal(out=rstd2[:], in_=std2[:])
            nc.vector.tensor_tensor(out=coef[:, 0:1], in0=consts_sb[:, 2:3],
                                    in1=rstd2[:, 0:1], op=OP.mult)
            nc.vector.tensor_tensor(out=coef[:, 2:3], in0=consts_sb[:, 0:1],
                                    in1=rstd2[:, 1:2], op=OP.mult)
            mba = pers.tile([DIM, 2], f32)
            nc.vector.tensor_tensor(out=mba[:, 0:1], in0=tmp[:, 0:1],
                                    in1=coef[:, 0:1], op=OP.mult)
            nc.vector.tensor_tensor(out=mba[:, 1:2], in0=tmp[:, 2:3],
                                    in1=coef[:, 2:3], op=OP.mult)
            nc.vector.tensor_tensor(out=coef[:, 1:2], in0=consts_sb[:, 3:4],
                                    in1=mba[:, 0:1], op=OP.subtract)
            nc.vector.tensor_tensor(out=coef[:, 3:4], in0=consts_sb[:, 1:2],
                                    in1=mba[:, 1:2], op=OP.subtract)

            coef_dr = dram.tile([DIM, 4], f32)
            nc.sync.dma_start(out=coef_dr[:], in_=coef[:])
            crep = pers.tile([P, 4], f32)
            for b in range(4):
                nc.sync.dma_start(out=crep[32 * b:32 * b + 32, :], in_=coef_dr[:])

            nc.scalar.activation(out=den0[:], in_=hnew[:], func=AF.Relu,
                                 scale=crep[:, 2:3], bias=crep[:, 3:4])
            nc.sync.dma_start(out=h_out_T[:], in_=den0[:])

            # ================= pass 2: e_out =================
            for (w, nd_off, c_off, C) in tile_plan:
                e2 = sb.tile([P, MTC], f32, tag="et")
                nc.sync.dma_start(out=e2[:, :C], in_=eij_st[:, c_off:c_off + C])
                nc.scalar.activation(out=e2[:, :C], in_=e2[:, :C], func=AF.Relu,
                                     scale=crep[:, 0:1], bias=crep[:, 1:2])
                nc.sync.dma_start(out=e_out_T[:, c_off:c_off + C], in_=e2[:, :C])

    nc.compile()
    return nc


# ---------------------------------------------------------------------------
# reassembly
# ---------------------------------------------------------------------------

def reassemble(plan, reasm, results, cfg):
    h_out = np.zeros((cfg.n_nodes, DIM), np.float32)
    e_out = np.zeros((cfg.n_edges, DIM), np.float32)
    feat = np.arange(DIM)
    npc = cfg.nodes_per_core
    for k, r in enumerate(reasm):
        eT = np.asarray(results[k]["e_out_T"])
        hT = np.asarray(results[k]["h_out_T"])
        rows = 32 * r["node_blk"][:, None] + feat[None, :]
        h_out[r["lo"]:r["lo"] + npc] = hT[rows, r["node_col"][:, None]]
        rows = 32 * r["blk"][:, None] + feat[None, :]
        e_out[r["eid"]] = eT[rows, r["gcol"][:, None]]
    return h_out, e_out


# ---------------------------------------------------------------------------
# entry point
# ---------------------------------------------------------------------------

_CACHE = {}


def kernel(h, e, norm, src, dst, gamma_h, beta_h, gamma_e, beta_e,
           _cfg=None, _sim=False):
    cfg = _cfg or Cfg()
    plan, in_maps, reasm = host_prep(h, e, norm, src, dst, cfg)
    set_bn_params(in_maps,
                  np.asarray(gamma_h, np.float32), np.asarray(beta_h, np.float32),
                  np.asarray(gamma_e, np.float32), np.asarray(beta_e, np.float32))

    sig = (cfg.n_nodes, cfg.n_edges, cfg.gather_mode,
           tuple(sorted(plan["budgets"].items())))
    if sig in _CACHE:
        nc = _CACHE[sig]
    else:
        nc = build_program(plan)
        _CACHE[sig] = nc

    if _sim:
        import concourse.bass_interp as bass_interp
        sim = bass_interp.MultiCoreSim(nc, cfg.n_cores)
        for k in range(cfg.n_cores):
            for name, val in in_maps[k].items():
                sim.cores[k].tensor(name)[:] = val
        sim.simulate()
        results = [{nm: np.array(sim.cores[k].mem_tensor(nm))
                    for nm in ("e_out_T", "h_out_T")} for k in range(cfg.n_cores)]
    else:
        from concourse.bass_utils import run_bass_kernel_spmd
        res = run_bass_kernel_spmd(nc, in_maps, list(range(cfg.n_cores)))
        results = res.results

    return reassemble(plan, reasm, results, cfg)
